# revision 11
# baseline (speedup 1.0000x reference)
"""BBoxTargetExpand on 8 TRN2 NeuronCores.

The reference is `where(labels > 0, x, x)` for both float tensors — an
identity copy. So the device kernel is a pure HBM->HBM memcpy of the two
f32 tensors, sharded over rows across the 8 cores; `labels` never needs
to touch the device.
"""

import numpy as np

import concourse.bass as bass
import concourse.mybir as mybir
from concourse.bass_utils import run_bass_kernel_spmd

M = 8_000_000
N = 4
N_CORES = 8
M_SHARD = M // N_CORES          # 1_000_000 rows per core
ELEMS = M_SHARD * N             # 4_000_000 f32 = 16 MiB per tensor per core

_nc_cache = None


def _build():
    global _nc_cache
    if _nc_cache is not None:
        return _nc_cache
    nc = bass.Bass()
    t_in = nc.declare_dram_parameter("t_in", [ELEMS], mybir.dt.float32, isOutput=False)
    w_in = nc.declare_dram_parameter("w_in", [ELEMS], mybir.dt.float32, isOutput=False)
    t_out = nc.declare_dram_parameter("t_out", [ELEMS], mybir.dt.float32, isOutput=True)
    w_out = nc.declare_dram_parameter("w_out", [ELEMS], mybir.dt.float32, isOutput=True)

    with (
        nc.semaphore("dma_sem_t") as sem_t,
        nc.semaphore("dma_sem_w") as sem_w,
    ):
        # Two HWDGE rings: sync (qSyncDynamicHW) carries t, scalar
        # (qScalarDynamicHW) carries w, so every SDMA engine has two
        # independent descriptor streams to overlap (raises engine
        # occupancy ~82% -> ~95% vs issuing both copies from sync).
        # Straight-line (no nc.Block): drops the block-exit all-engine
        # barrier from the measured window (the NEFF epilogue emits its
        # own barrier + semaphore-restore chain regardless).
        # Issue AFTER the framework preamble (not hoisted above it): an
        # interleaved A/B showed starting the copy during the NEFF boot
        # phase slows the whole transfer by ~15 us.
        nc.sync.dma_start(out=t_out[:], in_=t_in[:]).then_inc(sem_t, 16)
        nc.scalar.dma_start(out=w_out[:], in_=w_in[:]).then_inc(sem_w, 16)
        nc.sync.wait_ge(sem_t, 16)
        nc.scalar.wait_ge(sem_w, 16)

    _nc_cache = nc
    return nc


def _run(bbox_targets, bbox_weights, **kwargs):
    nc = _build()
    t = np.ascontiguousarray(np.asarray(bbox_targets, dtype=np.float32)).reshape(
        N_CORES, ELEMS
    )
    w = np.ascontiguousarray(np.asarray(bbox_weights, dtype=np.float32)).reshape(
        N_CORES, ELEMS
    )
    in_maps = [{"t_in": t[c], "w_in": w[c]} for c in range(N_CORES)]
    res = run_bass_kernel_spmd(nc, in_maps, list(range(N_CORES)), **kwargs)
    t_out = np.concatenate(
        [res.results[c]["t_out"] for c in range(N_CORES)]
    ).reshape(M, N)
    w_out = np.concatenate(
        [res.results[c]["w_out"] for c in range(N_CORES)]
    ).reshape(M, N)
    return (t_out, w_out), res


def kernel(bbox_targets, bbox_weights, labels=None, **kwargs):
    (t_out, w_out), _ = _run(bbox_targets, bbox_weights)
    return (t_out, w_out)



# revision 12
# speedup vs baseline: 1.0388x; 1.0388x over previous
"""BBoxTargetExpand on 8 TRN2 NeuronCores.

The reference is `where(labels > 0, x, x)` for both float tensors — an
identity copy. So the device kernel is a pure HBM->HBM memcpy of the two
f32 tensors, sharded over rows across the 8 cores; `labels` never needs
to touch the device.
"""

import numpy as np

import concourse.bass as bass
import concourse.mybir as mybir
from concourse.bass_utils import run_bass_kernel_spmd

M = 8_000_000
N = 4
N_CORES = 8
M_SHARD = M // N_CORES          # 1_000_000 rows per core
ELEMS = M_SHARD * N             # 4_000_000 f32 = 16 MiB per tensor per core

_nc_cache = None


def _build():
    global _nc_cache
    if _nc_cache is not None:
        return _nc_cache
    nc = bass.Bass()
    t_in = nc.declare_dram_parameter("t_in", [ELEMS], mybir.dt.float32, isOutput=False)
    w_in = nc.declare_dram_parameter("w_in", [ELEMS], mybir.dt.float32, isOutput=False)
    t_out = nc.declare_dram_parameter("t_out", [ELEMS], mybir.dt.float32, isOutput=True)
    w_out = nc.declare_dram_parameter("w_out", [ELEMS], mybir.dt.float32, isOutput=True)

    with (
        nc.semaphore("dma_sem_t") as sem_t,
        nc.semaphore("dma_sem_w") as sem_w,
    ):
        # Two HWDGE rings: sync (qSyncDynamicHW) carries t, scalar
        # (qScalarDynamicHW) carries w, so every SDMA engine has two
        # independent descriptor streams to overlap (raises engine
        # occupancy ~82% -> ~95% vs issuing both copies from sync).
        # Straight-line (no nc.Block): drops the block-exit all-engine
        # barrier from the measured window (the NEFF epilogue emits its
        # own barrier + semaphore-restore chain regardless).
        nc.sync.dma_start(out=t_out[:], in_=t_in[:]).then_inc(sem_t, 16)
        nc.scalar.dma_start(out=w_out[:], in_=w_in[:]).then_inc(sem_w, 16)
        nc.sync.wait_ge(sem_t, 16)
        nc.scalar.wait_ge(sem_w, 16)

    # Hoist the two DMA issues to the top of the block, ahead of the
    # framework register moves / const-AP memsets / init barrier. The
    # copies depend on none of that (DRAM params are bound and semaphores
    # zeroed at NEFF load), and InstDrain does not block on in-flight
    # DMA, so the transfer starts ~1.5 us earlier. 7 interleaved A/B
    # rounds vs the unhoisted order showed parity-or-better timing.
    insts = nc.m.functions[0].blocks[0].instructions
    dmas = [i for i in insts if type(i).__name__ == "InstDMACopy"]
    for d in dmas:
        insts.remove(d)
    for j, d in enumerate(dmas):
        insts.insert(j, d)

    _nc_cache = nc
    return nc


def _run(bbox_targets, bbox_weights, **kwargs):
    nc = _build()
    t = np.ascontiguousarray(np.asarray(bbox_targets, dtype=np.float32)).reshape(
        N_CORES, ELEMS
    )
    w = np.ascontiguousarray(np.asarray(bbox_weights, dtype=np.float32)).reshape(
        N_CORES, ELEMS
    )
    in_maps = [{"t_in": t[c], "w_in": w[c]} for c in range(N_CORES)]
    res = run_bass_kernel_spmd(nc, in_maps, list(range(N_CORES)), **kwargs)
    t_out = np.concatenate(
        [res.results[c]["t_out"] for c in range(N_CORES)]
    ).reshape(M, N)
    w_out = np.concatenate(
        [res.results[c]["w_out"] for c in range(N_CORES)]
    ).reshape(M, N)
    return (t_out, w_out), res


def kernel(bbox_targets, bbox_weights, labels=None, **kwargs):
    (t_out, w_out), _ = _run(bbox_targets, bbox_weights)
    return (t_out, w_out)

